# revision 6
# baseline (speedup 1.0000x reference)
"""Distributed multi-head attention kernel for 8 TRN2 NeuronCores — v3.

Problem: B=2, N=2048, C=768, H=12 heads of dim 64.
Sharding: core i owns batch i//4 and global heads {r, r+4, r+8}, r=i%4.

Query-half-major schedule: phase A computes all three heads' attention for
queries 0:1023, phase B for 1024:2047; context is redistributed by
AllToAll so core s ends up owning query stripes {s*128..} of both halves
and both batches, then projects locally.  Collectives fire much earlier
in every core's program than the per-head scheme, so inter-core dispatch
skew is paid while useful work remains.  Phase B's collective is split
(j0+j1, then j2) so the final tail is just two jc-blocks of outproj.

Per (head, phase): 16 kb steps of paired K=64 score matmuls (head dims
duplicated in partition halves 0:64/64:128 so two 512-query matmuls run
concurrently in the PE array), exp split between ACT (exact) and DVE
(Schraudolph int16-bitcast) per half-tile, PV (K=128, M=65 with a
trailing ones column producing the denominator row) interleaved behind
exp with a small lag.  Normalization for heads 0/1 uses a DRAM-round-trip
partition broadcast (cden copy on the idle GPSIMD engine, off the exp
queues); the phase-last head instead uses a K=1 ones-matmul broadcast on
the idle PE with per-half pipelining, shortening the pre-collective tail.
A dummy AllToAll at kernel start absorbs the collective-init barrier.
"""

import numpy as np
import ml_dtypes

import concourse.bass as bass
import concourse.mybir as mybir
import concourse.tile as tile
from concourse import bacc
from concourse.bass_utils import run_bass_kernel_spmd

B, N, C, H, HD = 2, 2048, 768, 12, 64
SCALE = HD ** -0.5          # 0.125
P = 128
CB = C // P                 # 6 channel blocks
KB = N // P                 # 16 key blocks
QCH = 512
HPC = 3                     # heads per core
NCORES = 8
VW = HPC * (HD + 1)         # 195
QP = 1024                   # queries per phase
RQ = QP // NCORES           # 128 queries per dest per phase
PVLAG = 3

f32 = mybir.dt.float32
bf16 = mybir.dt.bfloat16
i16 = mybir.dt.int16
Exp = mybir.ActivationFunctionType.Exp
Identity = mybir.ActivationFunctionType.Identity

# Schraudolph exp constants (bf16 bit space), scale folded in
LOG2E = 1.4426950408889634
SEXP_A = 128 * LOG2E * SCALE
SEXP_B = 127 * 128 - 4.7


def _body(nc, tc, xT, wqkT, wvT, woT, bo_d, out_d, dbg=None):
    with (
        tc.tile_pool(name="const", bufs=1) as constp,
        tc.tile_pool(name="big", bufs=1) as bigp,
        tc.tile_pool(name="esp", bufs=2 * KB + 2 * PVLAG + 2) as esp,
        tc.tile_pool(name="smallp", bufs=2) as smallp,
        tc.tile_pool(name="normp", bufs=1) as normp,
        tc.tile_pool(name="outp", bufs=3) as outp,
        tc.tile_pool(name="psS", bufs=4, space="PSUM") as psS,
        tc.tile_pool(name="psC", bufs=2, space="PSUM") as psC,
        tc.tile_pool(name="dram", bufs=1, space="DRAM") as dramp,
    ):
        # ---- dummy A2A first: absorbs CC-init barrier + core dispatch skew
        dummy_sb = constp.tile([8, 64], bf16, name="dummy_sb")
        nc.vector.memset(dummy_sb[:, :], 0.0)
        send_d = dramp.tile([NCORES, 1, 64], bf16, name="send_d")
        recv_d = dramp.tile([NCORES, 1, 64], bf16, name="recv_d")
        nc.sync.dma_start(send_d[:, 0, :], dummy_sb[:, :])
        nc.gpsimd.collective_compute(
            "AllToAll", mybir.AluOpType.bypass,
            replica_groups=[list(range(NCORES))],
            ins=[send_d.opt()], outs=[recv_d.opt()])

        # ---- load inputs ----
        xT_sb = [bigp.tile([P, N], bf16, name=f"xT_sb_{cb}") for cb in range(CB)]
        wqkT_sb = bigp.tile([P, CB * 384], bf16, name="wqkT_sb")
        wvT_sb = bigp.tile([P, CB * 192], bf16, name="wvT_sb")
        woT_sb = bigp.tile([P, CB * C], bf16, name="woT_sb")
        bo_sb = bigp.tile([P, CB], f32, name="bo_sb")
        ones_sb = constp.tile([P, 64], f32, name="ones_sb")
        nc.vector.memset(ones_sb[:, :], 1.0)
        warm_sb = constp.tile([P, 1], f32, name="warm_sb")
        nc.scalar.activation(warm_sb[0:1, :], ones_sb[0:1, 0:1], Exp, scale=SCALE)
        # weights on the ACT DMA ring, x on the sync ring: parallel transfers.
        # x lands qn-block-major so the first Q/K projection matmuls can
        # start after ~1/4 of x has arrived.
        for cb in range(CB):
            nc.scalar.dma_start(wqkT_sb[:, cb * 384:(cb + 1) * 384], wqkT[cb * P:(cb + 1) * P, :])
        for qn in range(4):
            for cb in range(CB):
                nc.sync.dma_start(
                    xT_sb[cb][:, qn * QCH:(qn + 1) * QCH],
                    xT[cb * P:(cb + 1) * P, qn * QCH:(qn + 1) * QCH])
        for cb in range(CB):
            nc.scalar.dma_start(wvT_sb[:, cb * 192:(cb + 1) * 192], wvT[cb * P:(cb + 1) * P, :])
        for cb in range(CB):
            nc.scalar.dma_start(woT_sb[:, cb * C:(cb + 1) * C], woT[cb * P:(cb + 1) * P, :])
            nc.scalar.dma_start(bo_sb[:, cb:cb + 1], bo_d[cb * P:(cb + 1) * P, :])

        # ---- PE warmup during loads: ramp HAM before projections ----
        wtile = constp.tile([P, 640], bf16, name="wtile")
        nc.vector.memset(wtile[:, :], 0.001)
        for w in range(10):
            wps = psS.tile([P, QCH], f32, name=f"wps_{w}", tag="psS")
            nc.tensor.matmul(wps[:, 0:QCH], lhsT=wtile[:, 0:128],
                             rhs=wtile[:, 128:640], start=True, stop=True)

        # ---- Q/K projections: per head j, psum = [q_j (rows 0:64) | k_j (64:128)]
        qT_sb = bigp.tile([P, HPC * N], bf16, name="qT_sb")
        kT_sb = bigp.tile([P, HPC * N], bf16, name="kT_sb")
        for j in range(HPC):
            for qn in range(4):
                ps = psS.tile([P, QCH], f32, name=f"pj_{j}_{qn}", tag="psS")
                for cb in range(CB):
                    nc.tensor.matmul(
                        ps[:, 0:QCH],
                        lhsT=wqkT_sb[:, cb * 384 + j * 128: cb * 384 + (j + 1) * 128],
                        rhs=xT_sb[cb][:, qn * QCH: (qn + 1) * QCH],
                        start=(cb == 0), stop=(cb == CB - 1),
                    )
                cols = slice(j * N + qn * QCH, j * N + (qn + 1) * QCH)
                nc.vector.tensor_copy(qT_sb[0:64, cols], ps[0:64, 0:QCH])
                nc.scalar.copy(kT_sb[64:128, cols], ps[64:128, 0:QCH])
            # duplicate head dims into the other partition half (row-tile pairing)
            blk = slice(j * N, (j + 1) * N)
            nc.sync.dma_start(qT_sb[64:128, blk], qT_sb[0:64, blk])
            nc.sync.dma_start(kT_sb[0:64, blk], kT_sb[64:128, blk])

        # ---- V projection (all key blocks upfront) ----
        v_sb = bigp.tile([P, KB * VW], bf16, name="v_sb")
        for nb in range(KB):
            ps = psS.tile([P, QCH], f32, name=f"vps_{nb}", tag="psS")
            for cb in range(CB):
                nc.tensor.matmul(
                    ps[:, 0:192],
                    lhsT=xT_sb[cb][:, nb * P:(nb + 1) * P],
                    rhs=wvT_sb[:, cb * 192:(cb + 1) * 192],
                    start=(cb == 0), stop=(cb == CB - 1),
                )
            vv = v_sb[:, nb * VW:(nb + 1) * VW].rearrange("p (h w) -> p h w", h=HPC)
            pp = ps[:, 0:192].rearrange("p (h w) -> p h w", h=HPC)
            nc.vector.tensor_copy(vv[:, :, 0:64], pp[:, :, :])
            nc.vector.memset(vv[:, :, 64:65], 1.0)

        # ---- attention, query-half-major ----
        ctxT_sb = [bigp.tile([64, HPC * QP], bf16, name=f"ctxT_sb_{ph}")
                   for ph in range(2)]
        # ctxTf: [128, jp(3) x u(2) x ph(2) x b(2) x RQ] assembled for outproj
        ctxTf_sb = bigp.tile([P, HPC * 8 * RQ], bf16, name="ctxTf_sb")
        db_t = [normp.tile([64, QP], f32, name=f"db_{i}") for i in range(2)]
        rb_t = [normp.tile([64, QP], f32, name=f"rb_{i}") for i in range(2)]

        def score_exp(j, ph, kb, es_list):
            kcol = slice(j * N + kb * P, j * N + (kb + 1) * P)
            es_pair = []
            for half in range(2):
                qc = 2 * ph + half
                ro = slice(half * 64, (half + 1) * 64)
                sps = psS.tile([P, QCH], f32, name=f"sps_{j}_{ph}_{kb}_{half}", tag="psS")
                nc.tensor.matmul(
                    sps[:, :],
                    lhsT=kT_sb[ro, kcol],
                    rhs=qT_sb[ro, j * N + qc * QCH: j * N + (qc + 1) * QCH],
                    start=True, stop=True,
                )
                if (kb + half) % 2 == 0:
                    es = esp.tile([P, QCH], bf16, name=f"esA_{j}_{ph}_{kb}_{half}", tag="es")
                    nc.scalar.activation(es, sps, Exp, scale=SCALE)
                    es_pair.append(es)
                else:
                    es16 = esp.tile([P, QCH], i16, name=f"esD_{j}_{ph}_{kb}_{half}", tag="es")
                    nc.vector.tensor_scalar(
                        out=es16[:, :], in0=sps[:, :],
                        scalar1=SEXP_A, scalar2=SEXP_B,
                        op0=mybir.AluOpType.mult, op1=mybir.AluOpType.add)
                    es_pair.append(es16.bitcast(bf16))
            es_list.append(es_pair)

        def pv_step(j, cps, kb, es_list, halves=(0, 1)):
            for half in halves:
                nc.tensor.matmul(
                    cps[0:65, half * QCH:(half + 1) * QCH],
                    lhsT=v_sb[:, kb * VW + j * 65: kb * VW + (j + 1) * 65],
                    rhs=es_list[kb][half][:, :],
                    start=(kb == 0), stop=(kb == KB - 1))

        def send_dma(ph, j, send_h, jo, slots):
            # slot s of send_h gets this core's context for dest s's query
            # stripe; row block jo*64 holds local head j
            for s in slots:
                nc.sync.dma_start(
                    send_h[s, jo * 64:(jo + 1) * 64, :],
                    ctxT_sb[ph][0:64, j * QP + s * RQ: j * QP + (s + 1) * RQ])

        def norm_send(j, ph, cps, send_h, jo):
            # heads 0/1: DRAM-round-trip partition broadcast; the cden copy
            # runs on DVE, whose next-head exp has PVLAG slack (ACT's gates
            # the next head's first PV)
            cden = smallp.tile([65, QP], f32, name=f"cden_{j}_{ph}", tag="rec")
            nc.vector.tensor_copy(cden[64:65, :], cps[64:65, :])
            rtmp = dramp.tile([1, QP], f32, name=f"rtmp_{j}_{ph}")
            nc.sync.dma_start(rtmp[:, :], cden[64:65, :])
            db = db_t[j % 2]
            rb = rb_t[j % 2]
            nc.sync.dma_start(db[0:64, :], rtmp[0:1, :].partition_broadcast(64))
            nc.vector.reciprocal_approx_fast(out=rb[0:64, :], in_=db[0:64, :])
            nc.vector.tensor_mul(
                ctxT_sb[ph][0:64, j * QP:(j + 1) * QP],
                cps[0:64, :], rb[0:64, :])
            send_dma(ph, j, send_h, jo, range(NCORES))

        def norm_half(j, ph, cps, send_h, jo, half):
            # phase-last head: ones-matmul broadcast on the (idle) PE
            cs = slice(half * QCH, (half + 1) * QCH)
            cden = smallp.tile([65, QCH], f32, name=f"cdh_{j}_{ph}_{half}", tag="rech")
            nc.scalar.copy(cden[64:65, :], cps[64:65, cs])
            bps = psS.tile([64, QCH], f32, name=f"bps_{j}_{ph}_{half}", tag="psS")
            nc.tensor.matmul(bps[0:64, :], lhsT=ones_sb[64:65, 0:64],
                             rhs=cden[64:65, :], start=True, stop=True)
            rb = rb_t[j % 2]
            nc.vector.reciprocal_approx_fast(out=rb[0:64, cs], in_=bps[0:64, :])
            nc.vector.tensor_mul(
                ctxT_sb[ph][0:64, j * QP + half * QCH: j * QP + (half + 1) * QCH],
                cps[0:64, cs], rb[0:64, cs])
            send_dma(ph, j, send_h, jo, range(4 * half, 4 * half + 4))

        def a2a(name, send_h, njp):
            recv_h = dramp.tile([NCORES, njp * 64, RQ], bf16, name=f"recv_{name}")
            nc.gpsimd.collective_compute(
                "AllToAll", mybir.AluOpType.bypass,
                replica_groups=[list(range(NCORES))],
                ins=[send_h.opt()], outs=[recv_h.opt()])
            return recv_h

        def scatter(ph, recv_h, jps, jo_of, rings):
            # grouped: source slots (2g, 2g+1) land in partitions 0:128 of
            # one [128, RQ] block; one DMA per (jp, group)
            n = 0
            for jp in jps:
                for g in range(4):
                    u, b = [0, 1, 0, 1][g], g // 2
                    co = jp * 8 * RQ + u * 4 * RQ + ph * 2 * RQ + b * RQ
                    src = recv_h[2 * g:2 * g + 2, jo_of(jp) * 64:(jo_of(jp) + 1) * 64, :]
                    rings[n % len(rings)].dma_start(ctxTf_sb[:, co:co + RQ], src)
                    n += 1

        send_A = dramp.tile([NCORES, HPC * 64, RQ], bf16, name="send_A")
        send_B1 = dramp.tile([NCORES, 2 * 64, RQ], bf16, name="send_B1")
        send_B2 = dramp.tile([NCORES, 64, RQ], bf16, name="send_B2")

        def run_head(j, ph, send_h, jo):
            last = (j == HPC - 1)
            es_list = []
            cps = psC.tile([65, QP], f32, name=f"cps_{j}_{ph}", tag="psC")
            for kb in range(KB):
                score_exp(j, ph, kb, es_list)
                if kb >= PVLAG:
                    pv_step(j, cps, kb - PVLAG, es_list, (0, 1) if not last else (0,))
            if not last:
                for kb in range(KB - PVLAG, KB):
                    pv_step(j, cps, kb, es_list)
                norm_send(j, ph, cps, send_h, jo)
            else:
                for kb in range(KB - PVLAG, KB):
                    pv_step(j, cps, kb, es_list, (0,))
                norm_half(j, ph, cps, send_h, jo, 0)
                for kb in range(KB):
                    pv_step(j, cps, kb, es_list, (1,))
                norm_half(j, ph, cps, send_h, jo, 1)

        # phase A
        for j in range(HPC):
            run_head(j, 0, send_A, j)
        recv_A = a2a("A", send_A, HPC)
        # phase B: j0, j1 -> B1; j2 -> B2
        run_head(0, 1, send_B1, 0)
        run_head(1, 1, send_B1, 1)
        recv_B1 = a2a("B1", send_B1, 2)
        # scatter A on the sync+scalar rings: collective A is long done, so
        # no head-of-line wait blocks the j2-B sends queued behind these
        scatter(0, recv_A, range(HPC), lambda jp: jp, [nc.sync, nc.scalar])
        run_head(2, 1, send_B2, 0)
        recv_B2 = a2a("B2", send_B2, 1)
        scatter(1, recv_B1, range(2), lambda jp: jp, [nc.gpsimd])
        scatter(1, recv_B2, [2], lambda jp: 0, [nc.sync, nc.scalar])

        # ---- output projection ----
        def oproj(ops, ph, cbo, jcs, start, stop):
            for jc in jcs:
                jp, u = divmod(jc, 2)
                nc.tensor.matmul(
                    ops[:, :],
                    lhsT=woT_sb[:, jc * C + cbo * P: jc * C + (cbo + 1) * P],
                    rhs=ctxTf_sb[:, jp * 8 * RQ + u * 4 * RQ + ph * 2 * RQ:
                                 jp * 8 * RQ + u * 4 * RQ + (ph + 1) * 2 * RQ],
                    start=start and jc == jcs[0], stop=stop and jc == jcs[-1],
                )

        def store(ops, ph, cbo):
            osb = outp.tile([P, 2 * RQ], f32, name=f"osb_{ph}_{cbo}", tag="osb")
            nc.scalar.activation(osb, ops, Identity, bias=bo_sb[:, cbo:cbo + 1])
            nc.sync.dma_start(
                out_d[cbo * P:(cbo + 1) * P, ph * 2 * RQ:(ph + 1) * 2 * RQ], osb)

        # phase A: cbo-outer, psum rotation in psC (slots free after last cps)
        for cbo in range(CB):
            ops = psC.tile([P, 2 * RQ], f32, name=f"opsA_{cbo}", tag="psC")
            oproj(ops, 0, cbo, list(range(CB)), True, True)
            store(ops, 0, cbo)
        # phase B part 1 (jc 0..3 from B1): six live accumulators
        opsB = []
        for cbo in range(CB):
            pool, tag = (psS, "psS") if cbo < 4 else (psC, "psC")
            ops = pool.tile([P, 2 * RQ], f32, name=f"opsB_{cbo}", tag=tag)
            opsB.append(ops)
            oproj(ops, 1, cbo, [0, 1, 2, 3], True, False)
        # phase B part 2 (jc 4..5 from B2) + store
        for cbo in range(CB):
            oproj(opsB[cbo], 1, cbo, [4, 5], False, True)
            store(opsB[cbo], 1, cbo)

        if dbg is not None:
            nc.sync.dma_start(dbg["qT"][:, :], qT_sb[:, :])
            nc.sync.dma_start(dbg["kT"][:, :], kT_sb[:, :])
            nc.sync.dma_start(dbg["v"][:, :], v_sb[:, :])
            for ph in range(2):
                nc.sync.dma_start(dbg["ctxT"][:, ph * HPC * QP:(ph + 1) * HPC * QP],
                                  ctxT_sb[ph][:, :])
            nc.sync.dma_start(dbg["ctxTf"][:, :], ctxTf_sb[:, :])


def build(debug_outs=False):
    nc = bacc.Bacc("TRN2", target_bir_lowering=False, debug=False, num_devices=NCORES)
    xT = nc.dram_tensor("xT", [C, N], bf16, kind="ExternalInput").ap()
    wqkT = nc.dram_tensor("wqkT", [C, HPC * 128], bf16, kind="ExternalInput").ap()
    wvT = nc.dram_tensor("wvT", [C, HPC * HD], bf16, kind="ExternalInput").ap()
    woT = nc.dram_tensor("woT", [C, C], bf16, kind="ExternalInput").ap()
    bo_d = nc.dram_tensor("bo", [C, 1], f32, kind="ExternalInput").ap()
    out_d = nc.dram_tensor("out", [C, 4 * RQ], f32, kind="ExternalOutput").ap()
    dbg = None
    if debug_outs:
        dbg = {
            "qT": nc.dram_tensor("dbg_qT", [P, HPC * N], bf16, kind="ExternalOutput").ap(),
            "kT": nc.dram_tensor("dbg_kT", [P, HPC * N], bf16, kind="ExternalOutput").ap(),
            "v": nc.dram_tensor("dbg_v", [P, KB * VW], bf16, kind="ExternalOutput").ap(),
            "ctxT": nc.dram_tensor("dbg_ctxT", [64, 2 * HPC * QP], bf16, kind="ExternalOutput").ap(),
            "ctxTf": nc.dram_tensor("dbg_ctxTf", [P, HPC * 1024], bf16, kind="ExternalOutput").ap(),
        }
    with tile.TileContext(nc) as tc:
        _body(nc, tc, xT, wqkT, wvT, woT, bo_d, out_d, dbg)
    nc.compile()
    return nc


_NC = None


def _get_nc():
    global _NC
    if _NC is None:
        _NC = build()
    return _NC


def make_in_maps(x, Wq, Wk, Wv, Wo, bo):
    x = np.asarray(x, np.float32)
    woT = np.ascontiguousarray(np.asarray(Wo, np.float32).T).astype(ml_dtypes.bfloat16)
    bo_col = np.ascontiguousarray(np.asarray(bo, np.float32).reshape(C, 1))
    Wq = np.asarray(Wq, np.float32)
    Wk = np.asarray(Wk, np.float32)
    Wv = np.asarray(Wv, np.float32)
    in_maps = []
    for i in range(NCORES):
        b = i // 4
        r = i % 4
        heads = [r, r + 4, r + 8]
        # wqk columns per head block j: [q_hj (64) | k_hj (64)]
        blocks = []
        for h in heads:
            hs = slice(h * HD, (h + 1) * HD)
            blocks.append(Wq[hs])
            blocks.append(Wk[hs])
        wqk = np.concatenate(blocks, axis=0).T          # [768, 384]
        wv_rows = np.concatenate([Wv[h * HD:(h + 1) * HD] for h in heads], axis=0)
        in_maps.append({
            "xT": np.ascontiguousarray(x[b].T).astype(ml_dtypes.bfloat16),
            "wqkT": np.ascontiguousarray(wqk).astype(ml_dtypes.bfloat16),
            "wvT": np.ascontiguousarray(wv_rows.T).astype(ml_dtypes.bfloat16),
            "woT": woT,
            "bo": bo_col,
        })
    return in_maps


def unshard(results):
    out = np.empty((B, N, C), np.float32)
    for i, r in enumerate(results):
        o = r["out"]  # [768, 512]: cols [phA b0 | phA b1 | phB b0 | phB b1]
        out[0, i * RQ:(i + 1) * RQ, :] = o[:, 0 * RQ:1 * RQ].T
        out[1, i * RQ:(i + 1) * RQ, :] = o[:, 1 * RQ:2 * RQ].T
        out[0, QP + i * RQ:QP + (i + 1) * RQ, :] = o[:, 2 * RQ:3 * RQ].T
        out[1, QP + i * RQ:QP + (i + 1) * RQ, :] = o[:, 3 * RQ:4 * RQ].T
    return out


def kernel(x, Wq, Wk, Wv, Wo, bo):
    nc = _get_nc()
    in_maps = make_in_maps(x, Wq, Wk, Wv, Wo, bo)
    res = run_bass_kernel_spmd(nc, in_maps, core_ids=list(range(NCORES)))
    return unshard(res.results)


# revision 7
# speedup vs baseline: 1.0474x; 1.0474x over previous
"""Distributed multi-head attention kernel for 8 TRN2 NeuronCores — v3.

Problem: B=2, N=2048, C=768, H=12 heads of dim 64.
Sharding: core i owns batch i//4 and global heads {r, r+4, r+8}, r=i%4.

Query-half-major schedule: phase A computes all three heads' attention for
queries 0:1023, phase B for 1024:2047; context is redistributed by
AllToAll so core s ends up owning query stripes {s*128..} of both halves
and both batches, then projects locally.  Collectives fire much earlier
in every core's program than the per-head scheme, so inter-core dispatch
skew is paid while useful work remains.  Phase B's collective is split
(j0+j1, then j2) so the final tail is just two jc-blocks of outproj.

Per (head, phase): 16 kb steps of paired K=64 score matmuls (head dims
duplicated in partition halves 0:64/64:128 so two 512-query matmuls run
concurrently in the PE array), exp split between ACT (exact) and DVE
(Schraudolph int16-bitcast) per half-tile, PV (K=128, M=65 with a
trailing ones column producing the denominator row) interleaved behind
exp with a small lag.  Normalization for heads 0/1 uses a DRAM-round-trip
partition broadcast (cden copy on the idle GPSIMD engine, off the exp
queues); the phase-last head instead uses a K=1 ones-matmul broadcast on
the idle PE with per-half pipelining, shortening the pre-collective tail.
A dummy AllToAll at kernel start absorbs the collective-init barrier.
"""

import numpy as np
import ml_dtypes

import concourse.bass as bass
import concourse.mybir as mybir
import concourse.tile as tile
from concourse import bacc
from concourse.bass_utils import run_bass_kernel_spmd

B, N, C, H, HD = 2, 2048, 768, 12, 64
SCALE = HD ** -0.5          # 0.125
P = 128
CB = C // P                 # 6 channel blocks
KB = N // P                 # 16 key blocks
QCH = 512
HPC = 3                     # heads per core
NCORES = 8
VW = HPC * (HD + 1)         # 195
QP = 1024                   # queries per phase
RQ = QP // NCORES           # 128 queries per dest per phase
PVLAG = 3

f32 = mybir.dt.float32
bf16 = mybir.dt.bfloat16
i16 = mybir.dt.int16
Exp = mybir.ActivationFunctionType.Exp
Identity = mybir.ActivationFunctionType.Identity

# Schraudolph exp constants (bf16 bit space), scale folded in
LOG2E = 1.4426950408889634
SEXP_A = 128 * LOG2E * SCALE
SEXP_B = 127 * 128 - 4.7


def _body(nc, tc, xT, wqkT, wvT, woT, bo_d, out_d, dbg=None):
    with (
        tc.tile_pool(name="const", bufs=1) as constp,
        tc.tile_pool(name="big", bufs=1) as bigp,
        tc.tile_pool(name="esp", bufs=2 * KB + 2 * PVLAG + 2) as esp,
        tc.tile_pool(name="smallp", bufs=2) as smallp,
        tc.tile_pool(name="normp", bufs=1) as normp,
        tc.tile_pool(name="outp", bufs=3) as outp,
        tc.tile_pool(name="psS", bufs=4, space="PSUM") as psS,
        tc.tile_pool(name="psC", bufs=2, space="PSUM") as psC,
        tc.tile_pool(name="dram", bufs=1, space="DRAM") as dramp,
    ):
        # ---- dummy A2A first: absorbs CC-init barrier + core dispatch skew
        dummy_sb = constp.tile([8, 64], bf16, name="dummy_sb")
        nc.vector.memset(dummy_sb[:, :], 0.0)
        send_d = dramp.tile([NCORES, 1, 64], bf16, name="send_d")
        recv_d = dramp.tile([NCORES, 1, 64], bf16, name="recv_d")
        nc.sync.dma_start(send_d[:, 0, :], dummy_sb[:, :])
        nc.gpsimd.collective_compute(
            "AllToAll", mybir.AluOpType.bypass,
            replica_groups=[list(range(NCORES))],
            ins=[send_d.opt()], outs=[recv_d.opt()])

        # ---- load inputs ----
        xT_sb = [bigp.tile([P, N], bf16, name=f"xT_sb_{cb}") for cb in range(CB)]
        wqkT_sb = bigp.tile([P, CB * 384], bf16, name="wqkT_sb")
        wvT_sb = bigp.tile([P, CB * 192], bf16, name="wvT_sb")
        woT_sb = bigp.tile([P, CB * C], bf16, name="woT_sb")
        bo_sb = bigp.tile([P, CB], f32, name="bo_sb")
        ones_sb = constp.tile([P, 64], f32, name="ones_sb")
        nc.vector.memset(ones_sb[:, :], 1.0)
        warm_sb = constp.tile([P, 1], f32, name="warm_sb")
        nc.scalar.activation(warm_sb[0:1, :], ones_sb[0:1, 0:1], Exp, scale=SCALE)
        # weights on the ACT DMA ring, x on the sync ring: parallel transfers.
        # x lands qn-block-major so the first Q/K projection matmuls can
        # start after ~1/4 of x has arrived.
        for cb in range(CB):
            nc.scalar.dma_start(wqkT_sb[:, cb * 384:(cb + 1) * 384], wqkT[cb * P:(cb + 1) * P, :])
        for qn in range(4):
            for cb in range(CB):
                nc.sync.dma_start(
                    xT_sb[cb][:, qn * QCH:(qn + 1) * QCH],
                    xT[cb * P:(cb + 1) * P, qn * QCH:(qn + 1) * QCH])
        for cb in range(CB):
            nc.scalar.dma_start(wvT_sb[:, cb * 192:(cb + 1) * 192], wvT[cb * P:(cb + 1) * P, :])
        for cb in range(CB):
            nc.scalar.dma_start(woT_sb[:, cb * C:(cb + 1) * C], woT[cb * P:(cb + 1) * P, :])
            nc.scalar.dma_start(bo_sb[:, cb:cb + 1], bo_d[cb * P:(cb + 1) * P, :])

        # ---- PE warmup during loads: ramp HAM before projections ----
        wtile = constp.tile([P, 640], bf16, name="wtile")
        nc.vector.memset(wtile[:, :], 0.001)
        for w in range(24):
            wps = psS.tile([P, QCH], f32, name=f"wps_{w}", tag="psS")
            nc.tensor.matmul(wps[:, 0:QCH], lhsT=wtile[:, 0:128],
                             rhs=wtile[:, 128:640], start=True, stop=True)

        # ---- Q/K projections: per head j, psum = [q_j (rows 0:64) | k_j (64:128)]
        qT_sb = bigp.tile([P, HPC * N], bf16, name="qT_sb")
        kT_sb = bigp.tile([P, HPC * N], bf16, name="kT_sb")
        for j in range(HPC):
            for qn in range(4):
                ps = psS.tile([P, QCH], f32, name=f"pj_{j}_{qn}", tag="psS")
                for cb in range(CB):
                    nc.tensor.matmul(
                        ps[:, 0:QCH],
                        lhsT=wqkT_sb[:, cb * 384 + j * 128: cb * 384 + (j + 1) * 128],
                        rhs=xT_sb[cb][:, qn * QCH: (qn + 1) * QCH],
                        start=(cb == 0), stop=(cb == CB - 1),
                    )
                cols = slice(j * N + qn * QCH, j * N + (qn + 1) * QCH)
                nc.vector.tensor_copy(qT_sb[0:64, cols], ps[0:64, 0:QCH])
                nc.scalar.copy(kT_sb[64:128, cols], ps[64:128, 0:QCH])
            # duplicate head dims into the other partition half (row-tile pairing)
            blk = slice(j * N, (j + 1) * N)
            nc.sync.dma_start(qT_sb[64:128, blk], qT_sb[0:64, blk])
            nc.sync.dma_start(kT_sb[0:64, blk], kT_sb[64:128, blk])

        # ---- V projection (all key blocks upfront) ----
        v_sb = bigp.tile([P, KB * VW], bf16, name="v_sb")
        for nb in range(KB):
            ps = psS.tile([P, QCH], f32, name=f"vps_{nb}", tag="psS")
            for cb in range(CB):
                nc.tensor.matmul(
                    ps[:, 0:192],
                    lhsT=xT_sb[cb][:, nb * P:(nb + 1) * P],
                    rhs=wvT_sb[:, cb * 192:(cb + 1) * 192],
                    start=(cb == 0), stop=(cb == CB - 1),
                )
            vv = v_sb[:, nb * VW:(nb + 1) * VW].rearrange("p (h w) -> p h w", h=HPC)
            pp = ps[:, 0:192].rearrange("p (h w) -> p h w", h=HPC)
            nc.vector.tensor_copy(vv[:, :, 0:64], pp[:, :, :])
            nc.vector.memset(vv[:, :, 64:65], 1.0)

        # ---- attention, query-half-major ----
        ctxT_sb = [bigp.tile([64, HPC * QP], bf16, name=f"ctxT_sb_{ph}")
                   for ph in range(2)]
        # ctxTf: [128, jp(3) x u(2) x ph(2) x b(2) x RQ] assembled for outproj
        ctxTf_sb = bigp.tile([P, HPC * 8 * RQ], bf16, name="ctxTf_sb")
        db_t = [normp.tile([64, QP], f32, name=f"db_{i}") for i in range(2)]
        rb_t = [normp.tile([64, QP], f32, name=f"rb_{i}") for i in range(2)]

        def score_exp(j, ph, kb, es_list):
            kcol = slice(j * N + kb * P, j * N + (kb + 1) * P)
            es_pair = []
            for half in range(2):
                qc = 2 * ph + half
                ro = slice(half * 64, (half + 1) * 64)
                sps = psS.tile([P, QCH], f32, name=f"sps_{j}_{ph}_{kb}_{half}", tag="psS")
                nc.tensor.matmul(
                    sps[:, :],
                    lhsT=kT_sb[ro, kcol],
                    rhs=qT_sb[ro, j * N + qc * QCH: j * N + (qc + 1) * QCH],
                    start=True, stop=True,
                )
                if (kb + half) % 2 == 0:
                    es = esp.tile([P, QCH], bf16, name=f"esA_{j}_{ph}_{kb}_{half}", tag="es")
                    nc.scalar.activation(es, sps, Exp, scale=SCALE)
                    es_pair.append(es)
                else:
                    es16 = esp.tile([P, QCH], i16, name=f"esD_{j}_{ph}_{kb}_{half}", tag="es")
                    nc.vector.tensor_scalar(
                        out=es16[:, :], in0=sps[:, :],
                        scalar1=SEXP_A, scalar2=SEXP_B,
                        op0=mybir.AluOpType.mult, op1=mybir.AluOpType.add)
                    es_pair.append(es16.bitcast(bf16))
            es_list.append(es_pair)

        def pv_step(j, cps, kb, es_list, halves=(0, 1)):
            for half in halves:
                nc.tensor.matmul(
                    cps[0:65, half * QCH:(half + 1) * QCH],
                    lhsT=v_sb[:, kb * VW + j * 65: kb * VW + (j + 1) * 65],
                    rhs=es_list[kb][half][:, :],
                    start=(kb == 0), stop=(kb == KB - 1))

        def send_dma(ph, j, send_h, jo, slots):
            # slot s of send_h gets this core's context for dest s's query
            # stripe; row block jo*64 holds local head j
            for s in slots:
                nc.sync.dma_start(
                    send_h[s, jo * 64:(jo + 1) * 64, :],
                    ctxT_sb[ph][0:64, j * QP + s * RQ: j * QP + (s + 1) * RQ])

        def norm_send(j, ph, cps, send_h, jo):
            # heads 0/1: DRAM-round-trip partition broadcast; the cden copy
            # runs on DVE, whose next-head exp has PVLAG slack (ACT's gates
            # the next head's first PV)
            cden = smallp.tile([65, QP], f32, name=f"cden_{j}_{ph}", tag="rec")
            nc.vector.tensor_copy(cden[64:65, :], cps[64:65, :])
            rtmp = dramp.tile([1, QP], f32, name=f"rtmp_{j}_{ph}")
            nc.sync.dma_start(rtmp[:, :], cden[64:65, :])
            db = db_t[j % 2]
            rb = rb_t[j % 2]
            nc.sync.dma_start(db[0:64, :], rtmp[0:1, :].partition_broadcast(64))
            nc.vector.reciprocal_approx_fast(out=rb[0:64, :], in_=db[0:64, :])
            nc.vector.tensor_mul(
                ctxT_sb[ph][0:64, j * QP:(j + 1) * QP],
                cps[0:64, :], rb[0:64, :])
            send_dma(ph, j, send_h, jo, range(NCORES))

        def norm_half(j, ph, cps, send_h, jo, half):
            # phase-last head: ones-matmul broadcast on the (idle) PE
            cs = slice(half * QCH, (half + 1) * QCH)
            cden = smallp.tile([65, QCH], f32, name=f"cdh_{j}_{ph}_{half}", tag="rech")
            nc.scalar.copy(cden[64:65, :], cps[64:65, cs])
            bps = psS.tile([64, QCH], f32, name=f"bps_{j}_{ph}_{half}", tag="psS")
            nc.tensor.matmul(bps[0:64, :], lhsT=ones_sb[64:65, 0:64],
                             rhs=cden[64:65, :], start=True, stop=True)
            rb = rb_t[j % 2]
            nc.vector.reciprocal_approx_fast(out=rb[0:64, cs], in_=bps[0:64, :])
            nc.vector.tensor_mul(
                ctxT_sb[ph][0:64, j * QP + half * QCH: j * QP + (half + 1) * QCH],
                cps[0:64, cs], rb[0:64, cs])
            send_dma(ph, j, send_h, jo, range(4 * half, 4 * half + 4))

        def a2a(name, send_h, njp):
            recv_h = dramp.tile([NCORES, njp * 64, RQ], bf16, name=f"recv_{name}")
            nc.gpsimd.collective_compute(
                "AllToAll", mybir.AluOpType.bypass,
                replica_groups=[list(range(NCORES))],
                ins=[send_h.opt()], outs=[recv_h.opt()])
            return recv_h

        def scatter(ph, recv_h, jps, jo_of, rings):
            # grouped: source slots (2g, 2g+1) land in partitions 0:128 of
            # one [128, RQ] block; one DMA per (jp, group)
            n = 0
            for jp in jps:
                for g in range(4):
                    u, b = [0, 1, 0, 1][g], g // 2
                    co = jp * 8 * RQ + u * 4 * RQ + ph * 2 * RQ + b * RQ
                    src = recv_h[2 * g:2 * g + 2, jo_of(jp) * 64:(jo_of(jp) + 1) * 64, :]
                    rings[n % len(rings)].dma_start(ctxTf_sb[:, co:co + RQ], src)
                    n += 1

        send_A = dramp.tile([NCORES, HPC * 64, RQ], bf16, name="send_A")
        send_B1 = dramp.tile([NCORES, 2 * 64, RQ], bf16, name="send_B1")
        send_B2 = dramp.tile([NCORES, 64, RQ], bf16, name="send_B2")

        def run_head(j, ph, send_h, jo):
            last = (j == HPC - 1)
            es_list = []
            cps = psC.tile([65, QP], f32, name=f"cps_{j}_{ph}", tag="psC")
            for kb in range(KB):
                score_exp(j, ph, kb, es_list)
                if kb >= PVLAG:
                    pv_step(j, cps, kb - PVLAG, es_list, (0, 1) if not last else (0,))
            if not last:
                for kb in range(KB - PVLAG, KB):
                    pv_step(j, cps, kb, es_list)
                norm_send(j, ph, cps, send_h, jo)
            else:
                for kb in range(KB - PVLAG, KB):
                    pv_step(j, cps, kb, es_list, (0,))
                norm_half(j, ph, cps, send_h, jo, 0)
                for kb in range(KB):
                    pv_step(j, cps, kb, es_list, (1,))
                norm_half(j, ph, cps, send_h, jo, 1)

        # phase A
        for j in range(HPC):
            run_head(j, 0, send_A, j)
        recv_A = a2a("A", send_A, HPC)
        # phase B: j0, j1 -> B1; j2 -> B2
        run_head(0, 1, send_B1, 0)
        run_head(1, 1, send_B1, 1)
        recv_B1 = a2a("B1", send_B1, 2)
        # scatter A on the sync+scalar rings: collective A is long done, so
        # no head-of-line wait blocks the j2-B sends queued behind these
        scatter(0, recv_A, range(HPC), lambda jp: jp, [nc.sync, nc.scalar])
        run_head(2, 1, send_B2, 0)
        recv_B2 = a2a("B2", send_B2, 1)
        scatter(1, recv_B1, range(2), lambda jp: jp, [nc.gpsimd])
        scatter(1, recv_B2, [2], lambda jp: 0, [nc.sync, nc.scalar])

        # ---- output projection ----
        def oproj(ops, ph, cbo, jcs, start, stop):
            for jc in jcs:
                jp, u = divmod(jc, 2)
                nc.tensor.matmul(
                    ops[:, :],
                    lhsT=woT_sb[:, jc * C + cbo * P: jc * C + (cbo + 1) * P],
                    rhs=ctxTf_sb[:, jp * 8 * RQ + u * 4 * RQ + ph * 2 * RQ:
                                 jp * 8 * RQ + u * 4 * RQ + (ph + 1) * 2 * RQ],
                    start=start and jc == jcs[0], stop=stop and jc == jcs[-1],
                )

        def store(ops, ph, cbo):
            osb = outp.tile([P, 2 * RQ], f32, name=f"osb_{ph}_{cbo}", tag="osb")
            nc.scalar.activation(osb, ops, Identity, bias=bo_sb[:, cbo:cbo + 1])
            nc.sync.dma_start(
                out_d[cbo * P:(cbo + 1) * P, ph * 2 * RQ:(ph + 1) * 2 * RQ], osb)

        # phase A: cbo-outer, psum rotation in psC (slots free after last cps)
        for cbo in range(CB):
            ops = psC.tile([P, 2 * RQ], f32, name=f"opsA_{cbo}", tag="psC")
            oproj(ops, 0, cbo, list(range(CB)), True, True)
            store(ops, 0, cbo)
        # phase B part 1 (jc 0..3 from B1): six live accumulators
        opsB = []
        for cbo in range(CB):
            pool, tag = (psS, "psS") if cbo < 4 else (psC, "psC")
            ops = pool.tile([P, 2 * RQ], f32, name=f"opsB_{cbo}", tag=tag)
            opsB.append(ops)
            oproj(ops, 1, cbo, [0, 1, 2, 3], True, False)
        # phase B part 2 (jc 4..5 from B2) + store
        for cbo in range(CB):
            oproj(opsB[cbo], 1, cbo, [4, 5], False, True)
            store(opsB[cbo], 1, cbo)

        if dbg is not None:
            nc.sync.dma_start(dbg["qT"][:, :], qT_sb[:, :])
            nc.sync.dma_start(dbg["kT"][:, :], kT_sb[:, :])
            nc.sync.dma_start(dbg["v"][:, :], v_sb[:, :])
            for ph in range(2):
                nc.sync.dma_start(dbg["ctxT"][:, ph * HPC * QP:(ph + 1) * HPC * QP],
                                  ctxT_sb[ph][:, :])
            nc.sync.dma_start(dbg["ctxTf"][:, :], ctxTf_sb[:, :])


def build(debug_outs=False):
    nc = bacc.Bacc("TRN2", target_bir_lowering=False, debug=False, num_devices=NCORES)
    xT = nc.dram_tensor("xT", [C, N], bf16, kind="ExternalInput").ap()
    wqkT = nc.dram_tensor("wqkT", [C, HPC * 128], bf16, kind="ExternalInput").ap()
    wvT = nc.dram_tensor("wvT", [C, HPC * HD], bf16, kind="ExternalInput").ap()
    woT = nc.dram_tensor("woT", [C, C], bf16, kind="ExternalInput").ap()
    bo_d = nc.dram_tensor("bo", [C, 1], f32, kind="ExternalInput").ap()
    out_d = nc.dram_tensor("out", [C, 4 * RQ], f32, kind="ExternalOutput").ap()
    dbg = None
    if debug_outs:
        dbg = {
            "qT": nc.dram_tensor("dbg_qT", [P, HPC * N], bf16, kind="ExternalOutput").ap(),
            "kT": nc.dram_tensor("dbg_kT", [P, HPC * N], bf16, kind="ExternalOutput").ap(),
            "v": nc.dram_tensor("dbg_v", [P, KB * VW], bf16, kind="ExternalOutput").ap(),
            "ctxT": nc.dram_tensor("dbg_ctxT", [64, 2 * HPC * QP], bf16, kind="ExternalOutput").ap(),
            "ctxTf": nc.dram_tensor("dbg_ctxTf", [P, HPC * 1024], bf16, kind="ExternalOutput").ap(),
        }
    with tile.TileContext(nc) as tc:
        _body(nc, tc, xT, wqkT, wvT, woT, bo_d, out_d, dbg)
    nc.compile()
    return nc


_NC = None


def _get_nc():
    global _NC
    if _NC is None:
        _NC = build()
    return _NC


def make_in_maps(x, Wq, Wk, Wv, Wo, bo):
    x = np.asarray(x, np.float32)
    woT = np.ascontiguousarray(np.asarray(Wo, np.float32).T).astype(ml_dtypes.bfloat16)
    bo_col = np.ascontiguousarray(np.asarray(bo, np.float32).reshape(C, 1))
    Wq = np.asarray(Wq, np.float32)
    Wk = np.asarray(Wk, np.float32)
    Wv = np.asarray(Wv, np.float32)
    in_maps = []
    for i in range(NCORES):
        b = i // 4
        r = i % 4
        heads = [r, r + 4, r + 8]
        # wqk columns per head block j: [q_hj (64) | k_hj (64)]
        blocks = []
        for h in heads:
            hs = slice(h * HD, (h + 1) * HD)
            blocks.append(Wq[hs])
            blocks.append(Wk[hs])
        wqk = np.concatenate(blocks, axis=0).T          # [768, 384]
        wv_rows = np.concatenate([Wv[h * HD:(h + 1) * HD] for h in heads], axis=0)
        in_maps.append({
            "xT": np.ascontiguousarray(x[b].T).astype(ml_dtypes.bfloat16),
            "wqkT": np.ascontiguousarray(wqk).astype(ml_dtypes.bfloat16),
            "wvT": np.ascontiguousarray(wv_rows.T).astype(ml_dtypes.bfloat16),
            "woT": woT,
            "bo": bo_col,
        })
    return in_maps


def unshard(results):
    out = np.empty((B, N, C), np.float32)
    for i, r in enumerate(results):
        o = r["out"]  # [768, 512]: cols [phA b0 | phA b1 | phB b0 | phB b1]
        out[0, i * RQ:(i + 1) * RQ, :] = o[:, 0 * RQ:1 * RQ].T
        out[1, i * RQ:(i + 1) * RQ, :] = o[:, 1 * RQ:2 * RQ].T
        out[0, QP + i * RQ:QP + (i + 1) * RQ, :] = o[:, 2 * RQ:3 * RQ].T
        out[1, QP + i * RQ:QP + (i + 1) * RQ, :] = o[:, 3 * RQ:4 * RQ].T
    return out


def kernel(x, Wq, Wk, Wv, Wo, bo):
    nc = _get_nc()
    in_maps = make_in_maps(x, Wq, Wk, Wv, Wo, bo)
    res = run_bass_kernel_spmd(nc, in_maps, core_ids=list(range(NCORES)))
    return unshard(res.results)
